# revision 41
# baseline (speedup 1.0000x reference)
"""Trainium2 Bass kernel for nn_BandwidthPredictorNNHall.

Math: for each batch b (8 of them, one per NeuronCore) with particles
x [n=1024, d=4]:
    pilot_d = 1.0592 * std(x_d, ddof=1) * n^(-1/8)
    q = x / pilot,   K_ij = exp(-0.5 * |q_i - q_j|^2)
    s2_d = sum_ij K_ij ((q_jd - q_id)^2 - 1)
    s3-based bandwidth2 is exactly 0 (odd under i<->j) and treated as 0.
With Mp = [1, p, p^2] (n x 9, RAW particle units), every sum needed for
s2 is an entry of V = Mp^T K Mp; the host applies ~30 scalar flops.

Device pipeline per core (upper-triangle exp + transpose-free reduction;
engine-dispatch and PSUM-bank choices are trace-driven, see comments):
  - One contiguous 16KB input DMA [128, 8, 4]; particle order is
    irrelevant (all reductions are pair-permutation-invariant), so the
    fastest descriptor layout wins: particle 8i+t -> (partition i, tile t).
  - Stats (sum p, sum p^2) as two tiny PE accumulation groups feeding the
    var -> s2 = 1/pilot^2 chain (reciprocal only, no sqrt => a single
    {Exp, Copy} activation table load, warmed during the DMA wait).
  - Feature-major operands: 8 PE transposes stage p^T in PSUM; ScalarE +
    DVE split the PSUM->SBUF copy (QTrr, raw rhs); Pool builds the
    per-row-tile scaled lhsT slices QTls = (s2 * p)^T. G = QTls^T @ QTrr
    per row tile I over columns [128 I, 1024) in >=256-wide f32r chunks.
  - K'' = exp(G - r_i/2) for the upper-triangle blocks only: 589k exps
    instead of 1M (ScalarE is the stream bottleneck; the 8 activations
    run back-to-back). K''_ij = K_ij e^{+r_j/2}; the column scale is
    cancelled later by cneg = e^{-r/2} (bias row nhall = -r/2 comes from
    a diag(s2) rank-1 broadcast and one multiply/reduce).
  - Reduction with no PT transposes: per column tile J,
        psW[:, J, :] += matmul(lhsT=K''-block(I,J), rhs=Mp[I])
    accumulated over I (0.5*Mp for the diagonal block), one contiguous
    PSUM group per J (interleaved groups corrupt a bank; sequential
    region-groups are safe). After group J closes: wsb_J = psW_J * cneg_J
    (cancels the column scale exactly), Vs += wsb_J^T Mp[J]. By block
    symmetry of the true K, V = S + S^T with S = Vs (host, f64).
  - Tail latency: the blocks touching row tiles 5-7 ship to the host raw
    (two [128, 384] K'' DMAs, issued right after act5/act7) so every
    device W-group closes at act4; the 9x9 S DMA then overlaps the block
    transfers. The host folds the 6 remaining blocks in f64.
"""

import sys

sys.path.insert(0, "/opt/trn_rl_repo")

import numpy as np

_B, _N, _D = 8, 1024, 4
_P = 128
_NT = _N // _P  # 8 row tiles
_NM = 1 + 2 * _D  # 9 basis columns: [1, p, p^2]
_INV_SQRT_2PI = 1.0 / np.sqrt(2.0 * np.pi)
_RK = 0.282095
_FACT = 1.0592 * float(_N) ** (-1.0 / (4 + _D))

# row tile I covers columns [128*I, 1024); KT column offset per tile
_W = [_N - _P * i for i in range(_NT)]
_OFF = [0] * _NT
for _i in range(1, _NT):
    _OFF[_i] = _OFF[_i - 1] + _W[_i - 1]
_KTW = sum(_W)  # 4608

_NC = None  # compiled Bass module cache


def _build_kernel():
    import concourse.bass as bass  # noqa: F401
    import concourse.tile as tile
    from concourse import bacc, mybir
    from concourse.masks import make_identity

    f32 = mybir.dt.float32
    fr = mybir.dt.float32r
    Act = mybir.ActivationFunctionType
    Alu = mybir.AluOpType
    Ax = mybir.AxisListType

    nc = bacc.Bacc("TRN2", target_bir_lowering=False, debug=False, num_devices=_B)
    p_in = nc.dram_tensor("p", [_N, _D], f32, kind="ExternalInput")
    s_out = nc.dram_tensor("sout", [_NM, _NM], f32, kind="ExternalOutput")
    var_out = nc.dram_tensor("varout", [_D, 1], f32, kind="ExternalOutput")
    kd_out = nc.dram_tensor("kdout", [_P, 3 * _P], f32, kind="ExternalOutput")
    kd2_out = nc.dram_tensor("kd2out", [_P, 3 * _P], f32, kind="ExternalOutput")

    with tile.TileContext(nc) as tc:
        with (
            tc.tile_pool(name="singles", bufs=1) as singles,
            tc.tile_pool(name="psBig", bufs=2, space="PSUM") as psBig,
            tc.tile_pool(name="psW", bufs=1, space="PSUM") as psWp,
            tc.tile_pool(name="psV", bufs=1, space="PSUM") as psVp,
            tc.tile_pool(name="psLate", bufs=2, space="PSUM") as psLate,
        ):
            # ---- input DMA first: contiguous 16KB, particle (8i+r) ->
            # (partition i, slot r); any particle<->(tile,lane) bijection
            # is valid for the pairwise sums.
            mstat = singles.tile([_P, _NT, _D], f32, tag="mstat")
            nc.sync.dma_start(
                out=mstat, in_=p_in[:].rearrange("(i r) d -> i r d", i=_P)
            )

            # ---- constants (Pool/DVE, overlap the DMA wait)
            ident128 = singles.tile([_P, _P], f32, tag="identf")
            make_identity(nc, ident128)
            ones128 = singles.tile([_P, 1], f32, tag="ones128")
            nc.vector.memset(ones128, 1.0)
            half4 = singles.tile([_D, _P], f32, tag="half4")
            nc.vector.memset(half4, 0.5)
            mtall = singles.tile([_P, _NT, _NM], f32, tag="mtall")
            nc.vector.memset(mtall[:, :, 0:1], 1.0)
            # dummy Exp so the activation-table load runs during the DMA wait
            warm = singles.tile([1, 1], f32, tag="warm")
            nc.scalar.activation(out=warm, in_=ones128[0:1, 0:1], func=Act.Exp)

            # ---- squares
            msq = singles.tile([_P, _NT, _D], f32, tag="msq")
            nc.vector.tensor_mul(msq, mstat, mstat)

            # ---- stats on PE: psS[:,0] = sum p, psS[:,1] = sum p^2
            # (regions of the psV bank; all groups in this bank run
            # sequentially: psS x2, psbc, then Vs0..Vs7)
            psVm = psVp.tile([_P, 16], f32, tag="psvm")
            psS = psVm[0:_D, 9:11]
            for t in range(_NT):
                nc.tensor.matmul(
                    psS[:, 0:1], lhsT=mstat[:, t, :], rhs=ones128,
                    start=(t == 0), stop=(t == _NT - 1), skip_group_check=True,
                )
            for t in range(_NT):
                nc.tensor.matmul(
                    psS[:, 1:2], lhsT=msq[:, t, :], rhs=ones128,
                    start=(t == 0), stop=(t == _NT - 1), skip_group_check=True,
                )
            # feature-major rhs staging: 8 transposes into one 2-bank tile
            psQa = psLate.tile([_P, 512], f32, name="psQa", tag="psl")
            psQb = psLate.tile([_P, 512], f32, name="psQb", tag="psl")
            for c in range(_NT):
                dst = psQa if c < 4 else psQb
                nc.tensor.transpose(
                    dst[0:_D, (c % 4) * _P : (c % 4 + 1) * _P],
                    mstat[:, c, :], ident128,
                )

            # ---- var chain (DVE): s2col = 1/pilot^2 as a [4,1] column
            sums = singles.tile([_D, 2], f32, tag="sums")
            nc.vector.tensor_copy(sums, psS)
            t1 = singles.tile([_D, 1], f32, tag="t1")
            nc.vector.tensor_scalar(
                out=t1, in0=sums[:, 0:1], scalar1=sums[:, 0:1],
                scalar2=-1.0 / _N, op0=Alu.mult, op1=Alu.mult,
            )
            den = singles.tile([_D, 1], f32, tag="den")
            nc.vector.tensor_add(den, t1, sums[:, 1:2])  # (n-1) var
            var_t = singles.tile([_D, 1], f32, tag="var_t")
            nc.vector.tensor_scalar_mul(var_t, den, 1.0 / (_N - 1))
            nc.sync.dma_start(out=var_out[:], in_=var_t)
            denf = singles.tile([_D, 1], f32, tag="denf")
            nc.vector.tensor_scalar_mul(denf, den, _FACT * _FACT / (_N - 1))
            s2col = singles.tile([_D, 1], f32, tag="s2col")
            nc.vector.reciprocal(s2col, denf)  # 1/pilot^2

            # ---- raw rhs QTrr = p^T [4, 1024]: split the PSUM->SBUF copy
            # across ScalarE and DVE (no stats dependency -> early)
            # ---- nhall = -r/2 per particle: diag(s2) via one DVE op, one
            # rank-1 PE broadcast, then multiply/reduce
            diag4 = singles.tile([_D, _D], f32, tag="diag4")
            nc.vector.tensor_scalar_mul(diag4, ident128[0:_D, 0:_D], s2col)
            psbc = psVm[:, 12:16]
            nc.tensor.matmul(
                psbc, lhsT=half4, rhs=diag4, start=True, stop=True,
                skip_group_check=True,
            )  # 0.5 * s2_d broadcast to all partitions
            scr = singles.tile([_P, _NT, _D], f32, tag="scr")
            nc.vector.tensor_mul(
                scr, msq, psbc.unsqueeze(1).broadcast_to((_P, _NT, _D))
            )
            nhall = singles.tile([_P, _NT], f32, tag="nhall")
            nc.vector.tensor_reduce(
                out=nhall, in_=scr, axis=Ax.X, op=Alu.add, negate=True
            )
            cneg = singles.tile([_P, _NT], f32, tag="cneg")
            nc.scalar.activation(out=cneg, in_=nhall, func=Act.Exp)

            # ---- raw rhs QTrr = p^T [4, 1024]: split the PSUM->SBUF copy
            # across ScalarE and DVE
            QTrr = singles.tile([_D, _N], fr, tag="qtrr")
            nc.scalar.copy(QTrr[:, 0:_P], psQa[0:_D, 0:_P])
            nc.scalar.copy(QTrr[:, _P:512], psQa[0:_D, _P:512])
            nc.vector.tensor_copy(QTrr[:, 512:_N], psQb[0:_D, :])

            # ---- scaled lhsT slices from SBUF QTrr on the otherwise-idle
            # Pool engine; Mp tiles [1 | p | p^2] and mthalf
            QTls = singles.tile([_D, _NT, _P], fr, tag="qtls")
            for c in range(2):
                nc.gpsimd.tensor_scalar_mul(
                    QTls[:, c, :], QTrr[:, c * _P : (c + 1) * _P], s2col
                )
            nc.gpsimd.tensor_copy(mtall[:, :, 1 : 1 + _D], mstat)
            nc.gpsimd.tensor_copy(mtall[:, :, 1 + _D : _NM], msq)
            mthalf = singles.tile([_P, _NT, _NM], f32, tag="mthalf")
            nc.gpsimd.tensor_scalar_mul(mthalf, mtall, 0.5)
            for c in range(2, _NT):
                nc.gpsimd.tensor_scalar_mul(
                    QTls[:, c, :], QTrr[:, c * _P : (c + 1) * _P], s2col
                )

            # ---- main stream: per row tile I (ascending), Gram chunks for
            # columns [128I, 1024) -> one exp -> per-block W matmuls; the
            # W group J closes at I==J, then its Vs contribution fires.
            KT = singles.tile([_P, _KTW], f32, tag="kt")
            psW = psWp.tile([_P, _NT, _NM], f32, tag="psw")
            psVs = psVm[0:_NM, 0:_NM]
            wsb = singles.tile([_P, _NT, _NM], f32, tag="wsb")
            psg_t = [None] * _NT
            psg_base = [0, 0, 0, 0, 512, 640, 768, 896]

            def g_chunks(i):
                cs = _P * i
                if cs < 512:
                    return [(cs, 512), (512, _N)]
                return [(cs, _N)]

            def emit_g(i):
                # G7 (128 cols) rides in the tail of G5's tile with
                # start=False: G5's bank-zeroing start clears its region,
                # so the accumulate lands on zeros. Avoids a psLate
                # buffer-rotation WAR that would stall act7 on act5.
                if i < 4:
                    psg = psBig.tile([_P, _N], f32, tag="psg")
                elif i == 7:
                    # a 5th psBig tile (bank of G2, free after act2): avoids
                    # the tile-WAR on act5/act6 that a psLate slot would add
                    psg = psBig.tile([_P, _P], f32, name="psg7", tag="psg")
                else:
                    psg = psLate.tile(
                        [_P, _N - psg_base[i]], f32, name=f"psl{i}", tag="psl"
                    )
                psg_t[i] = psg
                for a, b in g_chunks(i):
                    nc.tensor.matmul(
                        psg[:, a - psg_base[i] : b - psg_base[i]],
                        lhsT=QTls[:, i, :],
                        rhs=QTrr[:, a:b],
                        start=True, stop=True, skip_group_check=True,
                    )

            def emit_w(j, i_last=None):
                # group J = blocks (I, J) for I <= i_last, contiguous in the
                # psW bank (interleaved PSUM groups corrupt it). For J >= 6
                # the host folds in the blocks touching row tiles 6/7 from
                # the kdout DMA, so those groups close at act5.
                if i_last is None:
                    i_last = j
                for i in range(i_last + 1):
                    rhs = mthalf[:, i, :] if i == j else mtall[:, i, :]
                    nc.tensor.matmul(
                        psW[:, j, :],
                        lhsT=KT[:, _OFF[i] + _P * (j - i) : _OFF[i] + _P * (j - i + 1)],
                        rhs=rhs,
                        start=(i == 0), stop=(i == i_last), skip_group_check=True,
                    )

            def emit_wsb(j, eng=None):
                # e^{-r_j/2} per partition cancels the K'' column scale
                if eng == "act":
                    nc.scalar.mul(wsb[:, j, :], psW[:, j, :], cneg[:, j : j + 1])
                else:
                    nc.vector.tensor_scalar_mul(
                        wsb[:, j, :], psW[:, j, :], cneg[:, j : j + 1]
                    )

            def emit_vsm(j):
                nc.tensor.matmul(
                    psVs, lhsT=wsb[:, j, :], rhs=mtall[:, j, :],
                    start=(j == 0), stop=(j == _NT - 1),
                )

            emit_g(0)
            emit_g(1)
            for i in range(_NT):
                cs = _P * i
                nc.scalar.activation(
                    out=KT[:, _OFF[i] : _OFF[i] + _W[i]],
                    in_=psg_t[i][:, cs - psg_base[i] : _N - psg_base[i]],
                    func=Act.Exp,
                    bias=nhall[:, i : i + 1],
                )
                if i + 2 < _NT:
                    emit_g(i + 2)
                if i < _NT - 3:
                    emit_w(i)
                    emit_wsb(i)
                    if i == _NT - 4:
                        for j in range(_NT - 3, _NT):
                            emit_w(j, _NT - 4)
                        nc.vector.tensor_mul(
                            wsb[:, _NT - 3 : _NT, :],
                            psW[:, _NT - 3 : _NT, :],
                            cneg[:, _NT - 3 : _NT]
                            .unsqueeze(2)
                            .broadcast_to((_P, 3, _NM)),
                        )
                        for j in range(_NT - 5, _NT):
                            emit_vsm(j)
                # Vs matmuls deferred one iteration so they don't clog the
                # 4-deep PE wait queue while their wsb is pending
                if 0 < i < _NT - 4:
                    emit_vsm(i - 1)
                if i == _NT - 3:
                    # row-5 K'' blocks can ship as soon as act5 lands
                    nc.sync.dma_start(
                        out=kd_out[:],
                        in_=KT[:, _OFF[_NT - 3] : _OFF[_NT - 3] + 3 * _P],
                    )
            nc.sync.dma_start(
                out=kd2_out[:], in_=KT[:, _OFF[_NT - 2] : _OFF[_NT - 2] + 3 * _P]
            )

            Vt = singles.tile([_NM, _NM], f32, tag="vt")
            nc.vector.tensor_copy(Vt, psVs)

            nc.sync.dma_start(out=s_out[:], in_=Vt)

    nc.compile()
    return nc


def _get_nc():
    global _NC
    if _NC is None:
        _NC = _build_kernel()
    return _NC


def finalize(S, var, kd, kd2, p):
    """Host-side tail: S [9,9] (V = S + S^T, raw-p units) plus the three
    offloaded K'' blocks (6,6),(6,7),(7,7) in kd [128, 384], var [4] ->
    bandwidth [4]."""
    S = S.astype(np.float64)
    var64 = var.astype(np.float64).reshape(_D)
    pilot64 = _FACT * np.sqrt(var64)
    p = p.astype(np.float64)
    kd = np.concatenate([kd.astype(np.float64), kd2.astype(np.float64)], axis=1)
    # device layout: (partition i, slot t) = particle 8i+t; kd holds the
    # K'' blocks of row tiles 5..7 (columns j >= 128*row), column-scaled
    # by e^{+r_j/2}
    tl = {}
    for t in range(5, 8):
        pt = p[t::8]
        qt = pt / pilot64
        tl[t] = (
            pt,
            np.exp(-0.5 * (qt * qt).sum(1)),
            np.concatenate([np.ones((_P, 1)), pt, pt * pt], axis=1),
        )
    col = 0
    for ti in range(5, 8):
        Mi = tl[ti][2]
        for tj in range(ti, 8):
            _, cj, Mj = tl[tj]
            Kb = kd[:, col : col + _P] * cj[None, :]
            col += _P
            C = Mi.T @ Kb @ Mj
            S = S + (0.5 * C if ti == tj else C)
    V = S + S.T
    var = var.astype(np.float64).reshape(_D)
    pilot = _FACT * np.sqrt(var)
    d = np.arange(_D)
    s2 = (
        (V[0, 5 + d] + V[5 + d, 0] - 2.0 * V[1 + d, 1 + d]) / pilot**2 - V[0, 0]
    ) * _INV_SQRT_2PI
    denom = _N * (_N - 1)
    I2 = s2 / pilot**5 / denom
    J1 = _RK / I2
    base = J1 / _N
    return (np.sign(base) * np.abs(base) ** 0.2).astype(np.float32)


def kernel(particles, weights=None, **_unused):
    from concourse.bass_utils import run_bass_kernel_spmd

    particles = np.ascontiguousarray(np.asarray(particles), dtype=np.float32)
    assert particles.shape == (_B, _N, _D), particles.shape

    nc = _get_nc()
    in_maps = [{"p": particles[c]} for c in range(_B)]
    res = run_bass_kernel_spmd(nc, in_maps, list(range(_B)))

    out = np.empty((_B, _D), np.float32)
    for c in range(_B):
        out[c] = finalize(
            res.results[c]["sout"], res.results[c]["varout"],
            res.results[c]["kdout"], res.results[c]["kd2out"], particles[c],
        )
    return out
